# revision 11
# baseline (speedup 1.0000x reference)
"""Fused attention block (BERT-style, LRP-gamma forward) on 8 Trainium2 NeuronCores.

Decomposition: 24 (batch, head) pairs -> 3 pairs per core (core c: batch c//4,
heads 3*(c%4)..3*(c%4)+2). Output projection partial sums are ReduceScattered
within each 4-core batch group; each core finishes bias+residual+LayerNorm for
its sequence quarter.

Forward math (stop_gradient tricks in the reference only affect gradients):
  q/k/v = x @ W.T + b ;  scores = q k^T (no scaling) ; probs = softmax
  ctx = probs @ v ;  y = ctx @ (Wo + g*relu(Wo)).T + (bo + g*relu(bo)) + x
  out = (y - mean) / (std_ddof1 + 1e-12)

Precision: probs path (q/k projections, scores) in fp32 matmuls (exact);
value path (V, probs@V, out-projection) in float32r (11-bit mantissa) whose
~2e-4 relative error enters `out` only through the small attention
contribution (~5e-6 relative on out).
"""
import numpy as np
from contextlib import ExitStack

import concourse.bass as bass
import concourse.bacc as bacc
import concourse.mybir as mybir
from concourse import tile, masks
from concourse.bass_utils import run_bass_kernel_spmd

F32 = mybir.dt.float32
F32R = mybir.dt.float32r
AF = mybir.ActivationFunctionType
OP = mybir.AluOpType

B, S, H, NH, HD = 2, 2048, 768, 12, 64
G = 0.05
EPS = 1e-12
NCORES = 8
HPC = 3            # heads per core
DQ = HPC * HD      # 192 = per-core head-feature width
SQ = S // 4        # 512 = per-core output sequence quarter
NF = H // 128      # 6 f-chunks
NT = S // 128      # 16 t-tiles

_prog_cache = {}
LAST_EXEC_TIME_NS = None


def _build_program(trace_sim=False):
    nc = bacc.Bacc("TRN2", target_bir_lowering=False, debug=False, num_devices=NCORES)

    x_d = nc.dram_tensor("x", [S, H], F32, kind="ExternalInput")
    resid_d = nc.dram_tensor("resid", [SQ, H], F32, kind="ExternalInput")
    wqT_d = nc.dram_tensor("wqT", [H, DQ], F32, kind="ExternalInput")
    wkT_d = nc.dram_tensor("wkT", [H, DQ], F32, kind="ExternalInput")
    wvT_d = nc.dram_tensor("wvT", [H, DQ], F32, kind="ExternalInput")
    woT_d = nc.dram_tensor("woT", [DQ, H], F32, kind="ExternalInput")
    bq_d = nc.dram_tensor("bq", [DQ], F32, kind="ExternalInput")
    bk_d = nc.dram_tensor("bk", [DQ], F32, kind="ExternalInput")
    bv_d = nc.dram_tensor("bv", [DQ], F32, kind="ExternalInput")
    bo_d = nc.dram_tensor("bo", [H], F32, kind="ExternalInput")

    probs_d = nc.dram_tensor("probs_l", [HPC, S, S], F32, kind="ExternalOutput")
    out_d = nc.dram_tensor("out_l", [SQ, H], F32, kind="ExternalOutput")

    with tile.TileContext(nc, trace_sim=trace_sim) as tc:
        with ExitStack() as octx:
            # ---- persistent pools ----
            pers = octx.enter_context(tc.tile_pool(name="pers", bufs=1))

            ident = pers.tile([128, 128], F32)
            masks.make_identity(nc, ident[:])

            # q/k transposed activations: heads 0,1 stacked [128, S]; head 2 [64, S]
            qT01 = pers.tile([128, S], F32)
            qT2 = pers.tile([64, S], F32)
            kT01 = pers.tile([128, S], F32)
            kT2 = pers.tile([64, S], F32)
            qT01r = pers.tile([128, S], F32R)
            qT2r = pers.tile([64, S], F32R)
            kT01r = pers.tile([128, S], F32R)
            kT2r = pers.tile([64, S], F32R)
            recip_all = pers.tile([128, HPC * NT], F32)
            # V natural layout [t, d] as f32r; t-tile i at cols i*DQ, head h at +h*HD
            v_r = pers.tile([128, NT * DQ], F32R)
            # ctx^T per local head [64, S] f32r
            ctxh = [pers.tile([64, S], F32R, tag=f"ctxh{i}", name=f"ctxh{i}") for i in range(HPC)]
            # gamma-modified Wo^T, per dd-chunk of 64: chunk i at cols i*H
            pwoT = pers.tile([64, HPC * H], F32R)
            pbo_bc = pers.tile([128, H], F32)
            ones_col = pers.tile([1, 128], F32)
            nc.gpsimd.memset(ones_col[:], 1.0)

            bq01 = pers.tile([128, 1], F32)
            bq2 = pers.tile([64, 1], F32)
            bk01 = pers.tile([128, 1], F32)
            bk2 = pers.tile([64, 1], F32)
            nc.sync.dma_start(bq01[:], bq_d[0:128])
            nc.sync.dma_start(bq2[:], bq_d[128:DQ])
            nc.sync.dma_start(bk01[:], bk_d[0:128])
            nc.sync.dma_start(bk2[:], bk_d[128:DQ])
            bv_row = pers.tile([1, DQ], F32)
            nc.sync.dma_start(bv_row[:], bv_d[:])
            bo_row = pers.tile([1, H], F32)
            nc.sync.dma_start(bo_row[:], bo_d[:])

            # pbo = bo + G*relu(bo)
            pbo_row = pers.tile([1, H], F32)
            nc.vector.tensor_relu(pbo_row[:], bo_row[:])
            nc.vector.tensor_scalar(pbo_row[:], pbo_row[:], G, None, OP.mult)
            nc.vector.tensor_add(pbo_row[:], pbo_row[:], bo_row[:])

            # DRAM bounce tensors for the collective
            dram = octx.enter_context(tc.tile_pool(name="dram", bufs=1, space="DRAM"))
            y_part = dram.tile([S, H], F32)
            y_red = dram.tile([SQ, H], F32)

            # ---- phase 1+2: load x, transpose, weights, projections ----
            with ExitStack() as p2:
                sb2 = p2.enter_context(tc.tile_pool(name="sb2", bufs=1))
                xin_pool = p2.enter_context(tc.tile_pool(name="xin", bufs=2))
                proj_ps = p2.enter_context(tc.tile_pool(name="proj_ps", bufs=2, space="PSUM"))
                pt_ps = p2.enter_context(tc.tile_pool(name="pt_ps", bufs=2, space="PSUM"))
                v_ps = p2.enter_context(tc.tile_pool(name="v_ps", bufs=2, space="PSUM"))

                xT = sb2.tile([128, NF * S], F32)  # f-chunk j at cols j*S
                for st in range(NT):
                    xin = xin_pool.tile([128, H], F32, tag="xin")
                    nc.sync.dma_start(xin[:], x_d[st * 128:(st + 1) * 128, :])
                    for fb in range(NF):
                        p = pt_ps.tile([128, 128], F32, tag="ptp")
                        nc.tensor.transpose(p[:], xin[:, fb * 128:(fb + 1) * 128], ident[:])
                        nc.vector.tensor_copy(
                            xT[:, fb * S + st * 128: fb * S + (st + 1) * 128], p[:]
                        )

                # weight loads: wT [H, DQ] -> [128, NF*DQ] (f-chunk j at cols j*DQ)
                wq_sb = sb2.tile([128, NF * DQ], F32)
                wk_sb = sb2.tile([128, NF * DQ], F32)
                wv_sb = sb2.tile([128, NF * DQ], F32)
                for (wsb, wd) in [(wq_sb, wqT_d), (wk_sb, wkT_d), (wv_sb, wvT_d)]:
                    nc.sync.dma_start(
                        wsb[:].rearrange("p (c d) -> p c d", d=DQ),
                        wd[:].rearrange("(c p) d -> p c d", p=128),
                    )
                # woT [DQ, H] -> 3 chunks [64, H] at cols i*H
                wo_sb = sb2.tile([64, HPC * H], F32)
                for i in range(HPC):
                    nc.sync.dma_start(
                        wo_sb[:, i * H:(i + 1) * H], woT_d[i * 64:(i + 1) * 64, :]
                    )
                # pwoT = wo + G*relu(wo), written as f32r
                nc.vector.tensor_relu(pwoT[:], wo_sb[:])
                nc.vector.scalar_tensor_tensor(
                    pwoT[:], pwoT[:], G, wo_sb[:], OP.mult, OP.add
                )

                # q/k projections (fp32): out qT [d, s]
                for (wsb, dst01, dst2, dr01, dr2, b01, b2) in [
                    (wq_sb, qT01, qT2, qT01r, qT2r, bq01, bq2),
                    (wk_sb, kT01, kT2, kT01r, kT2r, bk01, bk2),
                ]:
                    for mt, rows in [(0, 128), (1, 64)]:
                        for half in range(2):
                            ps = proj_ps.tile([128, 1024], F32, tag="proj")
                            for f in range(NF):
                                for nn in range(2):
                                    nc.tensor.matmul(
                                        ps[0:rows, nn * 512:(nn + 1) * 512],
                                        wsb[:, f * DQ + mt * 128: f * DQ + mt * 128 + rows],
                                        xT[:, f * S + half * 1024 + nn * 512:
                                           f * S + half * 1024 + (nn + 1) * 512],
                                        start=(f == 0), stop=(f == NF - 1),
                                    )
                            dst = dst01 if mt == 0 else dst2
                            dstr = dr01 if mt == 0 else dr2
                            bias = b01 if mt == 0 else b2
                            nc.vector.tensor_scalar(
                                dst[0:rows, half * 1024:(half + 1) * 1024],
                                ps[0:rows, :], bias[0:rows, :], None, OP.add,
                            )
                            nc.vector.tensor_scalar(
                                dstr[0:rows, half * 1024:(half + 1) * 1024],
                                ps[0:rows, :], bias[0:rows, :], None, OP.add,
                            )

                # V projection (fp32 matmul, f32r result) with fused bias
                for tt in range(NT):
                    vp = v_ps.tile([128, DQ], F32, tag="vps")
                    for f in range(NF):
                        nc.tensor.matmul(
                            vp[:],
                            xT[:, f * S + tt * 128: f * S + (tt + 1) * 128],
                            wv_sb[:, f * DQ:(f + 1) * DQ],
                            start=(f == 0), stop=False,
                        )
                    nc.tensor.matmul(vp[:], ones_col[:], bv_row[:],
                                     start=False, stop=True)
                    nc.vector.tensor_copy(v_r[:, tt * DQ:(tt + 1) * DQ], vp[:])

            # ---- phase 3: attention per (local head, s-chunk) ----
            with ExitStack() as p3:
                prob_pool = p3.enter_context(tc.tile_pool(name="prob", bufs=3))
                et_pool = p3.enter_context(tc.tile_pool(name="ett", bufs=3))
                sc_ps = p3.enter_context(tc.tile_pool(name="sc_ps", bufs=2, space="PSUM"))
                st_ps = p3.enter_context(tc.tile_pool(name="st_ps", bufs=2, space="PSUM"))
                ctx_ps = p3.enter_context(tc.tile_pool(name="ctx_ps", bufs=2, space="PSUM"))
                small = p3.enter_context(tc.tile_pool(name="small", bufs=4))

                def qk_ap(t01, t2, l, c0, cl):
                    if l < 2:
                        return t01[64 * l:64 * (l + 1), c0:c0 + cl]
                    return t2[0:64, c0:c0 + cl]

                for p in range(HPC):
                    for c4 in range(4):
                        # probs path (fp32 scores; exact softmax written out)
                        for s_t in range(4):
                            st = c4 * 4 + s_t
                            prob = prob_pool.tile([128, S], F32, tag="prob")
                            sums = []
                            for th in range(2):
                                ps = sc_ps.tile([128, 1024], F32, tag="sc")
                                for nn in range(2):
                                    nc.tensor.matmul(
                                        ps[:, nn * 512:(nn + 1) * 512],
                                        qk_ap(qT01, qT2, p, st * 128, 128),
                                        qk_ap(kT01, kT2, p,
                                              th * 1024 + nn * 512, 512),
                                        start=True, stop=True,
                                    )
                                sm = small.tile([128, 1], F32, tag="sums")
                                nc.scalar.activation(
                                    prob[:, th * 1024:(th + 1) * 1024], ps[:],
                                    AF.Exp, accum_out=sm[:],
                                )
                                sums.append(sm)
                            ssum = small.tile([128, 1], F32, tag="ssum")
                            nc.vector.tensor_add(ssum[:], sums[0][:], sums[1][:])
                            rc = recip_all[:, p * NT + st: p * NT + st + 1]
                            nc.vector.reciprocal(rc, ssum[:])
                            nc.vector.tensor_scalar(prob[:], prob[:], rc,
                                                    None, OP.mult)
                            nc.sync.dma_start(
                                probs_d[p, st * 128:(st + 1) * 128, :], prob[:]
                            )

                        # PV path: scoresT (f32r) -> exp -> ctx^T accumulate.
                        # Unnormalized; the softmax 1/sum is applied per-head in
                        # the output projection (per-partition there).
                        cps = ctx_ps.tile([64, 512], F32, tag="ctx")
                        for tch in range(NT):
                            stp = st_ps.tile([128, 512], F32, tag="stp")
                            nc.tensor.matmul(
                                stp[:],
                                qk_ap(kT01r, kT2r, p, tch * 128, 128),
                                qk_ap(qT01r, qT2r, p, c4 * 512, 512),
                                start=True, stop=True,
                            )
                            ett = et_pool.tile([128, 512], F32R, tag="ett")
                            nc.scalar.activation(ett[:], stp[:], AF.Exp)
                            nc.tensor.matmul(
                                cps[:],
                                v_r[:, tch * DQ + p * HD: tch * DQ + (p + 1) * HD],
                                ett[:],
                                start=(tch == 0), stop=(tch == NT - 1),
                            )
                        nc.vector.tensor_copy(
                            ctxh[p][:, c4 * 512:(c4 + 1) * 512], cps[:]
                        )

            # ---- phase 4: output projection partials (f32r, per head + recip) ----
            with ExitStack() as p4:
                y_ps = p4.enter_context(tc.tile_pool(name="y_ps", bufs=4, space="PSUM"))
                ypool = p4.enter_context(tc.tile_pool(name="ypool", bufs=3))
                for st in range(NT):
                    ysb = ypool.tile([128, H], F32, tag="ysb")
                    for ch in range(HPC):
                        yp = y_ps.tile([128, H], F32, tag="yps")
                        for nn, nw in [(0, 512), (1, 256)]:
                            nc.tensor.matmul(
                                yp[:, nn * 512: nn * 512 + nw],
                                ctxh[ch][:, st * 128:(st + 1) * 128],
                                pwoT[:, ch * H + nn * 512: ch * H + nn * 512 + nw],
                                start=True, stop=True,
                            )
                        rc = recip_all[:, ch * NT + st: ch * NT + st + 1]
                        if ch == 0:
                            nc.vector.tensor_scalar(ysb[:], yp[:], rc, None, OP.mult)
                        else:
                            nc.vector.scalar_tensor_tensor(
                                ysb[:], yp[:], rc, ysb[:], OP.mult, OP.add
                            )
                    nc.sync.dma_start(y_part[st * 128:(st + 1) * 128, :], ysb[:])

            # ---- phase 5: reduce-scatter + residual + layernorm ----
            nc.gpsimd.collective_compute(
                "ReduceScatter",
                OP.add,
                replica_groups=[[0, 1, 2, 3], [4, 5, 6, 7]],
                ins=[y_part[:].opt()],
                outs=[y_red[:].opt()],
            )
            with ExitStack() as p5:
                fpool = p5.enter_context(tc.tile_pool(name="fpool", bufs=3))
                fs = p5.enter_context(tc.tile_pool(name="fs", bufs=4))
                f_ps = p5.enter_context(tc.tile_pool(name="f_ps", bufs=1, space="PSUM"))
                # pbo broadcast tile via ones-column outer product
                pb_ps = f_ps.tile([128, H], F32, tag="pbo")
                nc.tensor.matmul(pb_ps[:, 0:512], ones_col[:], pbo_row[:, 0:512],
                                 start=True, stop=True)
                nc.tensor.matmul(pb_ps[:, 512:H], ones_col[:], pbo_row[:, 512:H],
                                 start=True, stop=True)
                nc.vector.tensor_copy(pbo_bc[:], pb_ps[:])
                for i in range(SQ // 128):
                    yt = fpool.tile([128, H], F32, tag="yfin")
                    nc.sync.dma_start(yt[:], y_red[i * 128:(i + 1) * 128, :])
                    rt = fpool.tile([128, H], F32, tag="rfin")
                    nc.sync.dma_start(rt[:], resid_d[i * 128:(i + 1) * 128, :])
                    nc.vector.tensor_add(yt[:], yt[:], rt[:])
                    nc.vector.tensor_add(yt[:], yt[:], pbo_bc[:])
                    # mean
                    sm = fs.tile([128, 1], F32, tag="lsum")
                    nc.vector.reduce_sum(sm[:], yt[:], axis=mybir.AxisListType.X)
                    mean = fs.tile([128, 1], F32, tag="lmean")
                    nc.vector.tensor_scalar(mean[:], sm[:], 1.0 / H, None, OP.mult)
                    cent = fpool.tile([128, H], F32, tag="cent")
                    nc.vector.tensor_scalar(cent[:], yt[:], mean[:], None, OP.subtract)
                    sq = fpool.tile([128, H], F32, tag="sq")
                    nc.scalar.activation(sq[:], cent[:], AF.Square)
                    vs = fs.tile([128, 1], F32, tag="lvs")
                    nc.vector.reduce_sum(vs[:], sq[:], axis=mybir.AxisListType.X)
                    var = fs.tile([128, 1], F32, tag="lvar")
                    nc.vector.tensor_scalar(var[:], vs[:], 1.0 / (H - 1), None, OP.mult)
                    # std = sqrt(var) with one Newton step:
                    # s1 = 0.5*(s0 + var/s0)
                    s0 = fs.tile([128, 1], F32, tag="ls0")
                    nc.scalar.activation(s0[:], var[:], AF.Sqrt)
                    r0 = fs.tile([128, 1], F32, tag="lr0")
                    nc.vector.reciprocal(r0[:], s0[:])
                    t0 = fs.tile([128, 1], F32, tag="lt0")
                    nc.vector.tensor_mul(t0[:], var[:], r0[:])
                    s1 = fs.tile([128, 1], F32, tag="ls1")
                    nc.vector.tensor_add(s1[:], s0[:], t0[:])
                    nc.vector.tensor_scalar(s1[:], s1[:], 0.5, EPS, OP.mult, OP.add)
                    rstd = fs.tile([128, 1], F32, tag="lrstd")
                    nc.vector.reciprocal(rstd[:], s1[:])
                    outt = fpool.tile([128, H], F32, tag="outt")
                    nc.vector.tensor_scalar(outt[:], cent[:], rstd[:], None, OP.mult)
                    nc.sync.dma_start(out_d[i * 128:(i + 1) * 128, :], outt[:])

    nc.compile()
    return nc


def _get_program():
    if "nc" not in _prog_cache:
        _prog_cache["nc"] = _build_program()
    return _prog_cache["nc"]


def kernel(hidden_states, Wq, bq, Wk, bk, Wv, bv, Wo, bo):
    hidden_states = np.asarray(hidden_states, dtype=np.float32)
    Wq, bq = np.asarray(Wq, np.float32), np.asarray(bq, np.float32)
    Wk, bk = np.asarray(Wk, np.float32), np.asarray(bk, np.float32)
    Wv, bv = np.asarray(Wv, np.float32), np.asarray(bv, np.float32)
    Wo, bo = np.asarray(Wo, np.float32), np.asarray(bo, np.float32)

    nc = _get_program()

    in_maps = []
    for c in range(NCORES):
        b, q = c // 4, c % 4
        rs, re = DQ * q, DQ * q + DQ
        in_maps.append({
            "x": np.ascontiguousarray(hidden_states[b]),
            "resid": np.ascontiguousarray(hidden_states[b, SQ * q:SQ * (q + 1)]),
            "wqT": np.ascontiguousarray(Wq[rs:re].T),
            "wkT": np.ascontiguousarray(Wk[rs:re].T),
            "wvT": np.ascontiguousarray(Wv[rs:re].T),
            "woT": np.ascontiguousarray(Wo[:, rs:re].T),
            "bq": np.ascontiguousarray(bq[rs:re]),
            "bk": np.ascontiguousarray(bk[rs:re]),
            "bv": np.ascontiguousarray(bv[rs:re]),
            "bo": np.ascontiguousarray(bo),
        })

    res = run_bass_kernel_spmd(nc, in_maps, list(range(NCORES)))
    global LAST_EXEC_TIME_NS
    LAST_EXEC_TIME_NS = res.exec_time_ns

    out = np.empty((B, S, H), np.float32)
    probs = np.empty((B, NH, S, S), np.float32)
    for c in range(NCORES):
        b, q = c // 4, c % 4
        probs[b, HPC * q:HPC * q + HPC] = res.results[c]["probs_l"]
        out[b, SQ * q:SQ * (q + 1)] = res.results[c]["out_l"]
    return out, probs
